# revision 1
# baseline (speedup 1.0000x reference)
"""Causal self-attention (B=2, T=2048, C=2048, H=16) on 8 TRN2 NeuronCores.

Sharding: data-parallel over batch (2) x tensor-parallel over heads (4 groups
of 4 heads). Core c handles batch c//4, head group c%4. Each core computes
QKV projections for its heads, RoPE, causal flash-style attention, and a
partial output projection over its slice of Wproj's input dim; the host sums
the 4 partials per batch (the "all-reduce").

On-device layout (per core): everything transposed so no on-device transpose
is ever needed:
  - x^T (C, T) streamed in 512-col chunks, bf16
  - Q^T, K^T per head: (D=128 part, T free) from W_q^T-chunk.T @ x^T chunks
  - V: (T part, 512 free) from x^T-chunk.T @ W_v^T
  - scores transposed: S^T(s,q) = K^T-chunk.T @ Q^T block, softmax without
    max-subtraction (|scores*scale| < ~12 so exp is fp32-safe), key-dim sums
    via ones-vector matmul, normalization applied after P@V via per-column
    reciprocal broadcast.
  - proj: aout^T-chunk.T @ W_proj^T slices -> partial (T, C) fp32 out.
"""

import numpy as np
import ml_dtypes

import concourse.bass as bass
import concourse.mybir as mybir
import concourse.tile as tile
from concourse.bass_utils import run_bass_kernel_spmd

P = 128          # partitions
T = 2048         # sequence length
C = 2048         # model dim
D = 128          # head dim
HC = 4           # local heads per core
KT = 16          # contraction tiles (C / P)
NQ = 4           # 512-wide chunks (T / F)
NT = 16          # 128-wide t tiles (T / P)
F = 512          # free-dim chunk
SCALE = float(D) ** -0.5
NEG = -1.0e30
FP32 = mybir.dt.float32
BF16 = mybir.dt.bfloat16
BDT = ml_dtypes.bfloat16


def _split_multiwait(nc: bass.Bass):
    """This neuronxcc build allows at most one sync-wait per instruction
    (and none on InstDrain); Tile's vector-clock sem assignment freely emits
    several. Hoist excess waits onto standalone event-semaphore instructions
    inserted just before the owner on the same engine — identical semantics,
    the engine sequencer simply performs the waits one at a time."""
    for f in nc.m.functions:
        for b in f.blocks:
            insts = b.instructions
            idx = 0
            while idx < len(insts):
                inst = insts[idx]
                si = inst.sync_info
                waits = si.on_wait if si else None
                keep = 0 if isinstance(inst, mybir.InstDrain) else 1
                if waits and len(waits) > keep:
                    n_hoist = len(waits) - keep
                    hoist, rest = list(waits[:n_hoist]), list(waits[n_hoist:])
                    new = []
                    for w in hoist:
                        ev = mybir.InstEventSemaphore(
                            name=nc.get_next_instruction_name(),
                            ins=[],
                            outs=[],
                            sync_info=mybir.SyncInfo(on_wait=[w], on_update=[]),
                        )
                        ev.engine = inst.engine
                        nc.register_instruction(ev, overwrite=True)
                        new.append(ev)
                    si.on_wait.clear()
                    si.on_wait.extend(rest)
                    insts[idx:idx] = new
                    idx += len(new)
                idx += 1


def build_nc(reps: int = 1, loop_reps: int = 1, small_out: bool = False) -> bass.Bass:
    nc = bass.Bass()
    xT_d = nc.declare_dram_parameter("xT", [C, T], BF16, isOutput=False)
    wqT_d = nc.declare_dram_parameter("wqT", [C, HC * D], BF16, isOutput=False)
    wkT_d = nc.declare_dram_parameter("wkT", [C, HC * D], BF16, isOutput=False)
    wvT_d = nc.declare_dram_parameter("wvT", [C, HC * D], BF16, isOutput=False)
    wpT_d = nc.declare_dram_parameter("wpT", [HC * D, C], BF16, isOutput=False)
    cos_d = nc.declare_dram_parameter("cosT", [D // 2, T], FP32, isOutput=False)
    sin_d = nc.declare_dram_parameter("sinT", [D // 2, T], FP32, isOutput=False)
    mb_d = nc.declare_dram_parameter("maskbias", [NQ, P, F], BF16, isOutput=False)
    # small_out: timing-only variant — all 16 output DMAs alias one t-tile so
    # the per-call host<->device payload is tiny but on-device work is identical
    out_d = nc.declare_dram_parameter(
        "out", [P if small_out else T, C], FP32, isOutput=True
    )

    MULT = mybir.AluOpType.mult
    ADD = mybir.AluOpType.add
    EXP = mybir.ActivationFunctionType.Exp

    with tile.TileContext(nc) as tc:
        with (
            tc.tile_pool(name="weights", bufs=1) as wpool,
            tc.tile_pool(name="consts", bufs=1) as cpool,
            tc.tile_pool(name="qkv", bufs=1) as qkvpool,
        ):
            wq = wpool.tile([P, KT, HC * D], BF16, tag="wq")
            wk = wpool.tile([P, KT, HC * D], BF16, tag="wk")
            wv = wpool.tile([P, KT, HC * D], BF16, tag="wv")
            wp = wpool.tile([P, HC, C], BF16, tag="wp")
            # spread input DMAs across queues; wq first so phase 1 can start
            nc.sync.dma_start(wq[:], wqT_d[:, :].rearrange("(k p) n -> p k n", p=P))
            nc.gpsimd.dma_start(wk[:], wkT_d[:, :].rearrange("(k p) n -> p k n", p=P))
            nc.gpsimd.dma_start(wv[:], wvT_d[:, :].rearrange("(k p) n -> p k n", p=P))
            nc.scalar.dma_start(wp[:], wpT_d[:, :].rearrange("(h p) n -> p h n", p=P))

            cosT = cpool.tile([D // 2, T], FP32, tag="cos")
            sinT = cpool.tile([D // 2, T], FP32, tag="sin")
            mb = cpool.tile([P, NQ, F], BF16, tag="mb")
            ones = cpool.tile([P, 1], BF16, tag="ones")
            ones_row = cpool.tile([1, P], FP32, tag="ones_row")
            nc.scalar.dma_start(cosT[:], cos_d[:, :])
            nc.scalar.dma_start(sinT[:], sin_d[:, :])
            nc.scalar.dma_start(mb[:], mb_d[:, :, :].rearrange("r p n -> p r n"))
            nc.vector.memset(ones[:], 1.0)
            nc.vector.memset(ones_row[:], 1.0)

            import contextlib as _ctxlib
            for _rep in range(reps):
             with (tc.For_i(0, loop_reps, 1) if loop_reps > 1 else _ctxlib.nullcontext()):
              qT = qkvpool.tile([P, HC, T], BF16, tag="qT")
              kT = qkvpool.tile([P, HC, T], BF16, tag="kT")
              v_sb = qkvpool.tile([P, NT, HC * D], BF16, tag="v")
              aoutT = qkvpool.tile([P, HC, T], BF16, tag="aoutT")

              # ---------------- Phase 1: QKV projection + RoPE ----------------
              with (
                  tc.tile_pool(name="p1", bufs=2) as p1,
                  tc.tile_pool(name="p1ps", bufs=2, space="PSUM") as pp1,
              ):
                  for tq in range(NQ):
                      ts = slice(tq * F, (tq + 1) * F)
                      xt = p1.tile([P, KT, F], BF16, tag="xt")
                      nc.sync.dma_start(
                          xt[:], xT_d[:, :].rearrange("(k p) t -> p k t", p=P)[:, :, ts]
                      )
                      for h in range(HC):
                          hs = slice(h * D, (h + 1) * D)
                          for dst, w in ((qT, wq), (kT, wk)):
                              ps = pp1.tile([P, F], FP32, tag="pqk")
                              for k in range(KT):
                                  nc.tensor.matmul(
                                      ps[:],
                                      w[:, k, hs],
                                      xt[:, k, :],
                                      start=(k == 0),
                                      stop=(k == KT - 1),
                                  )
                              # RoPE (LLaMA half-split): two base-0 PSUM
                              # reads (SB+SB tensor_tensor needs equal base
                              # partitions), then cheap all-bf16 DVE ops
                              a1 = p1.tile([D // 2, F], FP32, tag="ropes1")
                              a2 = p1.tile([D // 2, F], FP32, tag="ropes2")
                              nc.vector.tensor_copy(a1[:], ps[0 : D // 2, :])
                              nc.vector.tensor_copy(a2[:], ps[D // 2 : P, :])
                              t1 = p1.tile([D // 2, F], FP32, tag="rt1")
                              t2 = p1.tile([D // 2, F], FP32, tag="rt2")
                              cs = cosT[:, ts]
                              sn = sinT[:, ts]
                              nc.vector.tensor_mul(t1[:], a1[:], cs)
                              nc.vector.tensor_mul(t2[:], a2[:], sn)
                              nc.vector.tensor_sub(dst[0 : D // 2, h, ts], t1[:], t2[:])
                              t3 = p1.tile([D // 2, F], FP32, tag="rt3")
                              t4 = p1.tile([D // 2, F], FP32, tag="rt4")
                              nc.vector.tensor_mul(t3[:], a1[:], sn)
                              nc.vector.tensor_mul(t4[:], a2[:], cs)
                              nc.vector.tensor_add(dst[D // 2 : P, h, ts], t3[:], t4[:])
                      for vt in range(F // P):
                          t_idx = tq * (F // P) + vt
                          vs = slice(vt * P, (vt + 1) * P)
                          psv = pp1.tile([P, HC * D], FP32, tag="pv")
                          for k in range(KT):
                              nc.tensor.matmul(
                                  psv[:],
                                  xt[:, k, vs],
                                  wv[:, k, :],
                                  start=(k == 0),
                                  stop=(k == KT - 1),
                              )
                          nc.vector.tensor_copy(v_sb[:, t_idx, :], psv[:])

              # ---------------- Phase 2: causal attention ----------------
              with (
                  tc.tile_pool(name="p2", bufs=3) as p2,
                  tc.tile_pool(name="p2ps", bufs=2, space="PSUM") as pp2,
                  tc.tile_pool(name="p2pss", bufs=3, space="PSUM") as pp2s,
              ):
                  for qb in range(NQ):
                      qs = slice(qb * F, (qb + 1) * F)
                      for h in range(HC):
                          hs = slice(h * D, (h + 1) * D)
                          ps_o = pp2.tile([P, F], FP32, tag="po")
                          ps_sum = pp2.tile([1, F], FP32, tag="sumrb")
                          n_st = 4 * qb + 4
                          for st in range(n_st):
                              ss = slice(st * P, (st + 1) * P)
                              ps_s = pp2s.tile([P, F], FP32, tag="ps")
                              nc.tensor.matmul(
                                  ps_s[:], kT[:, h, ss], qT[:, h, qs],
                                  start=True, stop=True,
                              )
                              pt = p2.tile([P, F], BF16, tag="pt")
                              r = st - 4 * qb
                              nc.scalar.activation(pt[:], ps_s[:], EXP, scale=SCALE)
                              if r >= 0:
                                  # zero out the above-diagonal region (bf16 4x)
                                  nc.vector.tensor_mul(pt[:], pt[:], mb[:, r, :])
                              nc.tensor.matmul(
                                  ps_sum[:], ones[:], pt[:],
                                  start=(st == 0), stop=(st == n_st - 1),
                              )
                              nc.tensor.matmul(
                                  ps_o[:], v_sb[:, st, hs], pt[:],
                                  start=(st == 0), stop=(st == n_st - 1),
                              )
                          rec = p2.tile([1, F], FP32, tag="rec")
                          nc.vector.reciprocal(rec[:], ps_sum[:])
                          # broadcast reciprocal across partitions via PE outer
                          # product (ones column x reciprocal row)
                          rb = pp2.tile([P, F], FP32, tag="sumrb")
                          # fp32 matmul broadcast: 4 cyc/row but only 16 of
                          # these; keeps full-precision denominators
                          nc.tensor.matmul(
                              rb[:], ones_row[:], rec[:], start=True, stop=True
                          )
                          # DVE may read only one PSUM operand; stage rb in SBUF
                          rb_sb = p2.tile([P, F], FP32, tag="rbsb")
                          nc.vector.tensor_copy(rb_sb[:], rb[:])
                          nc.vector.tensor_mul(aoutT[:, h, qs], ps_o[:], rb_sb[:])

              # ---------------- Phase 3: output projection ----------------
              with (
                  tc.tile_pool(name="p3", bufs=2) as p3,
                  tc.tile_pool(name="p3ps", bufs=2, space="PSUM") as pp3,
              ):
                  for t in range(NT):
                      tsl = slice(t * P, (t + 1) * P)
                      ps_p = pp3.tile([P, NQ, F], FP32, tag="pp")
                      for h in range(HC):
                          for n in range(NQ):
                              nc.tensor.matmul(
                                  ps_p[:, n, :],
                                  aoutT[:, h, tsl],
                                  wp[:, h, n * F : (n + 1) * F],
                                  start=(h == 0),
                                  stop=(h == HC - 1),
                              )
                      ob = p3.tile([P, NQ, F], FP32, tag="ob")
                      nc.vector.tensor_copy(ob[:], ps_p[:])
                      out_eng = nc.sync if t % 2 == 0 else nc.gpsimd
                      osl = slice(0, P) if small_out else tsl
                      out_eng.dma_start(
                          out_d[osl, :].rearrange("p (n f) -> p n f", f=F), ob[:]
                      )
    _split_multiwait(nc)
    return nc


_NC = None


def _get_nc():
    global _NC
    if _NC is None:
        _NC = build_nc()
    return _NC


def _make_in_maps(inputs=None, x=None, Wqkv=None, Wproj=None, start_pos=0):
    if inputs is not None:
        x, Wqkv, Wproj = inputs["x"], inputs["Wqkv"], inputs["Wproj"]
        start_pos = inputs.get("start_pos", 0)
    x = np.asarray(x)
    Wqkv = np.asarray(Wqkv)
    Wproj = np.asarray(Wproj)
    sp = int(np.asarray(start_pos))
    B = x.shape[0]

    half = D // 2
    inv_freq = 1.0 / (10000.0 ** (np.arange(half, dtype=np.float64) / half))
    pos = sp + np.arange(T, dtype=np.float64)
    ang = np.outer(inv_freq, pos)                      # (64, T)
    cosT = np.cos(ang).astype(np.float32)
    sinT = np.sin(ang).astype(np.float32)

    s_idx = np.arange(P)[:, None]
    q_idx = np.arange(F)[None, :]
    mb = np.empty((NQ, P, F), np.float32)
    for r in range(NQ):
        mb[r] = np.where(s_idx + P * r <= q_idx, 1.0, 0.0)
    mb = mb.astype(BDT)

    xTb = [np.ascontiguousarray(x[b].T).astype(BDT) for b in range(B)]
    wqT, wkT, wvT, wpT = [], [], [], []
    for g in range(4):
        rows = slice(512 * g, 512 * (g + 1))
        wqT.append(np.ascontiguousarray(Wqkv[rows, :].T).astype(BDT))
        wkT.append(np.ascontiguousarray(Wqkv[2048 + 512 * g : 2048 + 512 * (g + 1), :].T).astype(BDT))
        wvT.append(np.ascontiguousarray(Wqkv[4096 + 512 * g : 4096 + 512 * (g + 1), :].T).astype(BDT))
        wpT.append(np.ascontiguousarray(Wproj[:, rows].T).astype(BDT))

    in_maps = []
    for c in range(8):
        b, g = divmod(c, 4)
        in_maps.append(
            {
                "xT": xTb[b],
                "wqT": wqT[g],
                "wkT": wkT[g],
                "wvT": wvT[g],
                "wpT": wpT[g],
                "cosT": cosT,
                "sinT": sinT,
                "maskbias": mb,
            }
        )
    return in_maps


def kernel(x, Wqkv, Wproj, start_pos):
    x = np.asarray(x)
    B = x.shape[0]
    in_maps = _make_in_maps(x=x, Wqkv=Wqkv, Wproj=Wproj, start_pos=start_pos)
    res = run_bass_kernel_spmd(_get_nc(), in_maps, list(range(8))).results
    out = np.empty((B, T, C), np.float32)
    for b in range(B):
        acc = res[4 * b]["out"].astype(np.float32)
        for g in range(1, 4):
            acc = acc + res[4 * b + g]["out"]
        out[b] = acc
    return out



# revision 2
# speedup vs baseline: 3.3103x; 3.3103x over previous
"""Causal self-attention (B=2, T=2048, C=2048, H=16) on 8 TRN2 NeuronCores.

Sharding: data-parallel over batch (2) x tensor-parallel over heads (4 groups
of 4 heads). Core c handles batch c//4, head group c%4. Host sums the 4
partial projections per batch (the "all-reduce").

v2: single fused loop over 512-wide chunks (qkv+rope -> attention -> proj),
static PSUM bank plan (no per-phase pool churn), RoPE in 3 DVE tensor ops +
2 scalar-engine swap copies (sign-folded sin), PSUM->SBUF copies moved to the
scalar engine, causally-shortened diagonal score/sum/PV/exp widths, and the
reciprocal partition-broadcast matmul in fp32r instead of fp32.

On-device layout (per core), all transposed so no on-device transpose needed:
  - x^T (C, T) streamed in 512-col chunks, bf16
  - Q^T, K^T per head: (D=128 part, T free); V: (T part, 512 free)
  - scores transposed: S^T(s,q) = K^T-tile.T @ Q^T block; softmax without
    max-subtraction (|scores*scale| < ~12 so exp is fp32-safe); key-dim sums
    via ones-vector matmul; normalization applied after P@V via per-column
    reciprocal broadcast (fp32r PE outer product).
"""

import numpy as np
import ml_dtypes
import contextlib as _ctxlib

import concourse.bass as bass
import concourse.mybir as mybir
import concourse.tile as tile
from concourse.bass_utils import run_bass_kernel_spmd

P = 128          # partitions
T = 2048         # sequence length
C = 2048         # model dim
D = 128          # head dim
HC = 4           # local heads per core
KT = 16          # contraction tiles (C / P)
NQ = 4           # 512-wide chunks (T / F)
NT = 16          # 128-wide t tiles (T / P)
F = 512          # free-dim chunk
SCALE = float(D) ** -0.5
FP32 = mybir.dt.float32
F32R = mybir.dt.float32r
BF16 = mybir.dt.bfloat16
BDT = ml_dtypes.bfloat16


def _split_multiwait(nc: bass.Bass):
    """This neuronxcc build allows at most one sync-wait per instruction
    (and none on InstDrain); Tile's vector-clock sem assignment freely emits
    several. Hoist excess waits onto standalone event-semaphore instructions
    inserted just before the owner on the same engine — identical semantics,
    the engine sequencer simply performs the waits one at a time."""
    for f in nc.m.functions:
        for b in f.blocks:
            insts = b.instructions
            idx = 0
            while idx < len(insts):
                inst = insts[idx]
                si = inst.sync_info
                waits = si.on_wait if si else None
                keep = 0 if isinstance(inst, mybir.InstDrain) else 1
                if waits and len(waits) > keep:
                    n_hoist = len(waits) - keep
                    hoist, rest = list(waits[:n_hoist]), list(waits[n_hoist:])
                    new = []
                    for w in hoist:
                        ev = mybir.InstEventSemaphore(
                            name=nc.get_next_instruction_name(),
                            ins=[],
                            outs=[],
                            sync_info=mybir.SyncInfo(on_wait=[w], on_update=[]),
                        )
                        ev.engine = inst.engine
                        nc.register_instruction(ev, overwrite=True)
                        new.append(ev)
                    si.on_wait.clear()
                    si.on_wait.extend(rest)
                    insts[idx:idx] = new
                    idx += len(new)
                idx += 1


def build_nc(reps: int = 1, loop_reps: int = 1, small_out: bool = False) -> bass.Bass:
    nc = bass.Bass()
    xT_d = nc.declare_dram_parameter("xT", [C, T], BF16, isOutput=False)
    wqT_d = nc.declare_dram_parameter("wqT", [C, HC * D], BF16, isOutput=False)
    wkT_d = nc.declare_dram_parameter("wkT", [C, HC * D], BF16, isOutput=False)
    wvT_d = nc.declare_dram_parameter("wvT", [C, HC * D], BF16, isOutput=False)
    wpT_d = nc.declare_dram_parameter("wpT", [HC * D, C], BF16, isOutput=False)
    cs2_d = nc.declare_dram_parameter("cs2", [P, T], BF16, isOutput=False)
    sn2_d = nc.declare_dram_parameter("sn2", [P, T], BF16, isOutput=False)
    mbs_d = nc.declare_dram_parameter("mbs", [P, P], BF16, isOutput=False)
    out_d = nc.declare_dram_parameter(
        "out", [P if small_out else T, C], FP32, isOutput=True
    )

    EXP = mybir.ActivationFunctionType.Exp

    with tile.TileContext(nc) as tc:
        with (
            tc.tile_pool(name="weights", bufs=1) as wpool,
            tc.tile_pool(name="consts", bufs=1) as cpool,
            tc.tile_pool(name="kv", bufs=1) as kvpool,
            tc.tile_pool(name="qa", bufs=2) as qpool,
            tc.tile_pool(name="xls", bufs=2) as xpool,
            tc.tile_pool(name="rope", bufs=3) as rpool,
            tc.tile_pool(name="p2", bufs=3) as p2,
            tc.tile_pool(name="p2b", bufs=2) as p2b,
            tc.tile_pool(name="p3", bufs=2) as p3,
            tc.tile_pool(name="ppqk", bufs=2, space="PSUM") as ppqk,
            tc.tile_pool(name="ppv", bufs=1, space="PSUM") as ppv,
            tc.tile_pool(name="pps", bufs=2, space="PSUM") as pps,
            tc.tile_pool(name="ppo", bufs=2, space="PSUM") as ppo,
            tc.tile_pool(name="ppz", bufs=1, space="PSUM") as ppz,
        ):
            wq = wpool.tile([P, KT, HC * D], BF16, tag="wq")
            wk = wpool.tile([P, KT, HC * D], BF16, tag="wk")
            wv = wpool.tile([P, KT, HC * D], BF16, tag="wv")
            wp = wpool.tile([P, HC, C], BF16, tag="wp")
            nc.sync.dma_start(wq[:], wqT_d[:, :].rearrange("(k p) n -> p k n", p=P))
            nc.gpsimd.dma_start(wk[:], wkT_d[:, :].rearrange("(k p) n -> p k n", p=P))
            nc.gpsimd.dma_start(wv[:], wvT_d[:, :].rearrange("(k p) n -> p k n", p=P))
            nc.scalar.dma_start(wp[:], wpT_d[:, :].rearrange("(h p) n -> p h n", p=P))

            cs2 = cpool.tile([P, T], BF16, tag="cs2")
            sn2 = cpool.tile([P, T], BF16, tag="sn2")
            mbs = cpool.tile([P, P], BF16, tag="mbs")
            ones = cpool.tile([P, 1], BF16, tag="ones")
            ones_row_f = cpool.tile([1, P], FP32, tag="ones_row_f")
            ones_row = cpool.tile([1, P], F32R, tag="ones_row")
            nc.scalar.dma_start(cs2[:], cs2_d[:, :])
            nc.scalar.dma_start(sn2[:], sn2_d[:, :])
            nc.scalar.dma_start(mbs[:], mbs_d[:, :])
            nc.vector.memset(ones[:], 1.0)
            nc.vector.memset(ones_row_f[:], 1.0)
            with nc.allow_low_precision(reason="f32r ones for broadcast matmul"):
                nc.vector.tensor_copy(ones_row[:], ones_row_f[:])

            for _rep in range(reps):
             with (tc.For_i(0, loop_reps, 1) if loop_reps > 1 else _ctxlib.nullcontext()):
              kT = kvpool.tile([P, HC, T], BF16, tag="kT")
              v_sb = kvpool.tile([P, NT, HC * D], BF16, tag="v")
              for tq in range(NQ):
                  ts = slice(tq * F, (tq + 1) * F)
                  xt = xpool.tile([P, KT, F], BF16, tag="xt")
                  nc.sync.dma_start(
                      xt[:], xT_d[:, :].rearrange("(k p) t -> p k t", p=P)[:, :, ts]
                  )
                  qT = qpool.tile([P, HC, F], BF16, tag="qT")
                  aoT = qpool.tile([P, HC, F], BF16, tag="aoT")

                  # ---- QKV projection + RoPE for this 512-chunk ----
                  for h in range(HC):
                      hs = slice(h * D, (h + 1) * D)
                      for which, wgt in ((0, wq), (1, wk)):
                          ps = ppqk.tile([P, F], FP32, tag="pqk")
                          for k in range(KT):
                              nc.tensor.matmul(
                                  ps[:],
                                  wgt[:, k, hs],
                                  xt[:, k, :],
                                  start=(k == 0),
                                  stop=(k == KT - 1),
                              )
                          # RoPE (LLaMA half-split): sw = halves-swapped ps
                          # (scalar engine), then u = ps*cs2, w = sw*sn2 with
                          # sn2 = [-sin; sin], dst = u + w  (3 DVE ops).
                          sw = rpool.tile([P, F], BF16, tag="sw")
                          nc.scalar.copy(sw[0 : D // 2, :], ps[D // 2 : P, :])
                          nc.scalar.copy(sw[D // 2 : P, :], ps[0 : D // 2, :])
                          u = rpool.tile([P, F], BF16, tag="u")
                          nc.vector.tensor_mul(u[:], ps[:], cs2[:, ts])
                          w = rpool.tile([P, F], BF16, tag="w")
                          nc.vector.tensor_mul(w[:], sw[:], sn2[:, ts])
                          if which == 0:
                              nc.vector.tensor_add(qT[:, h, :], u[:], w[:])
                          else:
                              nc.vector.tensor_add(kT[:, h, ts], u[:], w[:])
                  for vt in range(F // P):
                      t_idx = tq * (F // P) + vt
                      vs = slice(vt * P, (vt + 1) * P)
                      pv = ppv.tile([P, HC * D], FP32, tag="pv")
                      for k in range(KT):
                          nc.tensor.matmul(
                              pv[:],
                              xt[:, k, vs],
                              wv[:, k, :],
                              start=(k == 0),
                              stop=(k == KT - 1),
                          )
                      nc.scalar.copy(v_sb[:, t_idx, :], pv[:])

                  # ---- causal attention for q-block tq ----
                  n_st = 4 * tq + 4
                  for h in range(HC):
                      hs = slice(h * D, (h + 1) * D)
                      po = ppo.tile([P, F], FP32, tag="po")
                      zs = ppz.tile([1, F], FP32, tag="zs")
                      for st in range(n_st):
                          ss = slice(st * P, (st + 1) * P)
                          r = st - 4 * tq
                          w0 = r * P if r > 0 else 0
                          qsl = slice(w0, F)
                          ps_s = pps.tile([P, F], FP32, tag="ps")
                          nc.tensor.matmul(
                              ps_s[:, qsl], kT[:, h, ss], qT[:, h, qsl],
                              start=True, stop=True,
                          )
                          pt = p2.tile([P, F], BF16, tag="pt")
                          nc.scalar.activation(pt[:, qsl], ps_s[:, qsl], EXP, scale=SCALE)
                          if r >= 0:
                              # zero the above-diagonal triangle of the 128-wide
                              # strip at q in [w0, w0+128)
                              nc.vector.tensor_mul(
                                  pt[:, w0 : w0 + P], pt[:, w0 : w0 + P], mbs[:]
                              )
                          nc.tensor.matmul(
                              zs[:, qsl], ones[:], pt[:, qsl],
                              start=(st == 0), stop=(st == n_st - 1),
                          )
                          nc.tensor.matmul(
                              po[:, qsl], v_sb[:, st, hs], pt[:, qsl],
                              start=(st == 0), stop=(st == n_st - 1),
                          )
                      rec = p2b.tile([1, F], F32R, tag="rec")
                      with nc.allow_low_precision(reason="f32r reciprocal feed"):
                          nc.vector.reciprocal(rec[:], zs[:])
                      # broadcast reciprocal across partitions via PE outer
                      # product (fp32r: full-rate rows vs fp32's 1/4)
                      rb = pps.tile([P, F], FP32, tag="ps")
                      nc.tensor.matmul(
                          rb[:], ones_row[:], rec[:], start=True, stop=True
                      )
                      rbs = p2b.tile([P, F], FP32, tag="rbs")
                      nc.scalar.copy(rbs[:], rb[:])
                      nc.vector.tensor_mul(aoT[:, h, :], po[:], rbs[:])

                  # ---- output projection for this chunk's 4 t-tiles ----
                  for vt in range(F // P):
                      t_idx = tq * (F // P) + vt
                      tsl = slice(vt * P, (vt + 1) * P)
                      ob = p3.tile([P, NQ, F], FP32, tag="ob")
                      for n in range(NQ):
                          pp = ppqk.tile([P, F], FP32, tag="pqk")
                          for h in range(HC):
                              nc.tensor.matmul(
                                  pp[:],
                                  aoT[:, h, tsl],
                                  wp[:, h, n * F : (n + 1) * F],
                                  start=(h == 0),
                                  stop=(h == HC - 1),
                              )
                          nc.scalar.copy(ob[:, n, :], pp[:])
                      out_eng = nc.sync if t_idx % 2 == 0 else nc.gpsimd
                      osl = (
                          slice(0, P)
                          if small_out
                          else slice(t_idx * P, (t_idx + 1) * P)
                      )
                      out_eng.dma_start(
                          out_d[osl, :].rearrange("p (n f) -> p n f", f=F), ob[:]
                      )
    _split_multiwait(nc)
    return nc


_NC = None


def _get_nc():
    global _NC
    if _NC is None:
        _NC = build_nc()
    return _NC


def _make_in_maps(inputs=None, x=None, Wqkv=None, Wproj=None, start_pos=0):
    if inputs is not None:
        x, Wqkv, Wproj = inputs["x"], inputs["Wqkv"], inputs["Wproj"]
        start_pos = inputs.get("start_pos", 0)
    x = np.asarray(x)
    Wqkv = np.asarray(Wqkv)
    Wproj = np.asarray(Wproj)
    sp = int(np.asarray(start_pos))
    B = x.shape[0]

    half = D // 2
    inv_freq = 1.0 / (10000.0 ** (np.arange(half, dtype=np.float64) / half))
    pos = sp + np.arange(T, dtype=np.float64)
    ang = np.outer(inv_freq, pos)                      # (64, T)
    cosT = np.cos(ang)
    sinT = np.sin(ang)
    cs2 = np.concatenate([cosT, cosT], axis=0).astype(BDT)      # (128, T)
    sn2 = np.concatenate([-sinT, sinT], axis=0).astype(BDT)     # (128, T)

    s_idx = np.arange(P)[:, None]
    q_idx = np.arange(P)[None, :]
    mbs = np.where(s_idx <= q_idx, 1.0, 0.0).astype(BDT)        # (128, 128)

    xTb = [np.ascontiguousarray(x[b].T).astype(BDT) for b in range(B)]
    wqT, wkT, wvT, wpT = [], [], [], []
    for g in range(4):
        rows = slice(512 * g, 512 * (g + 1))
        wqT.append(np.ascontiguousarray(Wqkv[rows, :].T).astype(BDT))
        wkT.append(np.ascontiguousarray(Wqkv[2048 + 512 * g : 2048 + 512 * (g + 1), :].T).astype(BDT))
        wvT.append(np.ascontiguousarray(Wqkv[4096 + 512 * g : 4096 + 512 * (g + 1), :].T).astype(BDT))
        wpT.append(np.ascontiguousarray(Wproj[:, rows].T).astype(BDT))

    in_maps = []
    for c in range(8):
        b, g = divmod(c, 4)
        in_maps.append(
            {
                "xT": xTb[b],
                "wqT": wqT[g],
                "wkT": wkT[g],
                "wvT": wvT[g],
                "wpT": wpT[g],
                "cs2": cs2,
                "sn2": sn2,
                "mbs": mbs,
            }
        )
    return in_maps


def kernel(x, Wqkv, Wproj, start_pos):
    x = np.asarray(x)
    B = x.shape[0]
    in_maps = _make_in_maps(x=x, Wqkv=Wqkv, Wproj=Wproj, start_pos=start_pos)
    res = run_bass_kernel_spmd(_get_nc(), in_maps, list(range(8))).results
    out = np.empty((B, T, C), np.float32)
    for b in range(B):
        acc = res[4 * b]["out"].astype(np.float32)
        for g in range(1, 4):
            acc = acc + res[4 * b + g]["out"]
        out[b] = acc
    return out
